# revision 6
# baseline (speedup 1.0000x reference)
"""NonLocalAttention2D Trainium2 kernel (v4).

Data-parallel over batch N=8: one image per NeuronCore.

Per-core math (x: (C=128, HW=4096) fp32):
  kv   = [Wv|Wk].T @ x              (80, 4096)  PE f32r (v rows 0:64, k 64:80)
  pool = maxpool2x2(kv)             (80, 1024)  DVE max chain -> kvh fp16
  A    = [Wq.T; Wq@bk].T @ [K; 1]   (128, 1024) PE fp16 (bias folded via ones
                                    row 80 of kvh), DVE copy -> ab f32
  bqk  = k.T @ bq, ebqk = exp(.)    (128, 8)    PE + ACT (bias bk.bq)
  vaugT= [vT*ebqk | ebqk]           (128, 8*65) PE transpose f16 + DVE -> bf16
  s_cb = ab_c.T @ x_b               (128k,512q) PE f32r -> psum
  attn = exp(s): tiles 0-2 ACT Exp; tile 3 DVE Schraudolph
         (int16(s*128/ln2 + 16252.5) bitcast bf16, ~2% rel err that cancels
         in the softmax normalization)
  av   = vaugT.T @ attn  (accum 8c) (65, 512)   PE bf16; row 64 = denom
  r    = recip_approx_fast(denom)   (1, 512)    DVE (input staged to SBUF)
  R65  = broadcast r over 65 parts  DRAM-bounce DMA (PE ones-matmul on tail)
  ao   = av * R65 (f32)             (65, 512)   DVE
  fin  = [g*Wo; g*bo'].T @ ao       (128, 512)  PE f32r
  out  = fin + x_b                  (128, 512)  DVE -> DMA out

vs v3: f32r matmuls read xf directly (no fp16 cast of x; ACT freed), the
exp stream is split ACT/DVE so neither paces the loop, biases fold into
the A matmul, and input DMAs dispatch from four engine queues in parallel.
"""

import sys

if "/opt/trn_rl_repo" not in sys.path:
    sys.path.insert(0, "/opt/trn_rl_repo")

import numpy as np

import concourse.bacc as bacc
import concourse.bass as bass
import concourse.tile as tile
from concourse import bass_utils, mybir

F32 = mybir.dt.float32
F32R = mybir.dt.float32r
F16 = mybir.dt.float16
BF16 = mybir.dt.bfloat16
I16 = mybir.dt.int16

C = 128          # channels
HW = 4096        # 64*64 pixels
L = 1024         # pooled keys (32*32)
D = 16           # attn dim
DV = 64          # value dim
KV = 80          # kv projection out width (v rows 0:64, k rows 64:80)
QB = 512         # q-block size
NB = HW // QB    # 8 q blocks
KC = 128         # keys per chunk
NCH = L // KC    # 8 key chunks
NCORES = 8
W16 = DV + C + 1 + C       # ident64 | wqt17 (rows 64:81) | bq | wfin
W32 = KV                   # wkv

# Schraudolph exp -> bf16 bits: bits = trunc(s * 2^7/ln2 + (127*2^7 - 4 + 0.5))
SCH_A = 128.0 / 0.6931471805599453
SCH_B = 16252.5


def build_kernel():
    nc = bacc.Bacc("TRN2", target_bir_lowering=False, debug=False)

    x_d = nc.dram_tensor("x", (C, HW), F32R, kind="ExternalInput").ap()
    wb16_d = nc.dram_tensor("wb16", (C, W16), F16, kind="ExternalInput").ap()
    wb32_d = nc.dram_tensor("wb32", (C, W32), F32R, kind="ExternalInput").ap()
    baux_d = nc.dram_tensor("baux", (C, 1), F32, kind="ExternalInput").ap()
    out_d = nc.dram_tensor("out", (C, HW), F32, kind="ExternalOutput").ap()

    from contextlib import ExitStack

    with tile.TileContext(nc) as tc, ExitStack() as ctx:
        singles = ctx.enter_context(tc.tile_pool(name="singles", bufs=1))
        s1_pool = ctx.enter_context(tc.tile_pool(name="s1", bufs=4))
        attn_pool = ctx.enter_context(tc.tile_pool(name="attn", bufs=2))
        r_pool = ctx.enter_context(tc.tile_pool(name="r", bufs=2))
        ao_pool = ctx.enter_context(tc.tile_pool(name="ao", bufs=2))
        out_pool = ctx.enter_context(tc.tile_pool(name="outp", bufs=3))
        dram_pool = ctx.enter_context(tc.tile_pool(name="dram", bufs=2, space="DRAM"))

        ps_sc = ctx.enter_context(tc.tile_pool(name="ps_sc", bufs=2, space="PSUM"))
        ps_av = ctx.enter_context(tc.tile_pool(name="ps_av", bufs=2, space="PSUM"))
        ps_fin = ctx.enter_context(tc.tile_pool(name="ps_fin", bufs=2, space="PSUM"))

        # ---- SBUF singles ----
        wb16 = singles.tile([C, W16], F16, tag="wb16")
        wb32 = singles.tile([C, W32], F32R, tag="wb32")
        xf = singles.tile([C, HW], F32R, tag="xf")
        xf32 = xf.bitcast(F32)
        kvh = singles.tile([KV + 1, L], F16, tag="kvh")  # v 0:64, k 64:80, ones 80
        ab = singles.tile([C, L], F32R, tag="ab")
        ones65 = singles.tile([1, DV + 1], BF16, tag="ones")
        baux = singles.tile([C, 1], F32, tag="baux")

        w_kv = wb32[:, 0:KV]
        identh = wb16[0:DV, 0:DV]
        w_qt17 = wb16[DV : KV + 1, DV : DV + C]     # rows 64:81
        b_q64 = wb16[DV : DV + D, DV + C : DV + C + 1]  # rows 64:80
        w_fin = wb16[0 : DV + 1, DV + C + 1 : DV + 2 * C + 1]
        bkbq = baux[:, 0:1]

        # ---- input DMAs: spread dispatch across the three DMA-capable
        # engine queues (sync, scalar/ACT, gpsimd) so the transfers start
        # as early and as parallel as possible. Piece 0 and the weights
        # gate the prologue -> they go first on each queue.
        nc.sync.dma_start(out=xf[:, 0:QB], in_=x_d[:, 0:QB])
        nc.scalar.dma_start(out=wb32, in_=wb32_d)
        nc.gpsimd.dma_start(out=xf[:, QB : 2 * QB], in_=x_d[:, QB : 2 * QB])
        nc.sync.dma_start(out=xf[:, 2 * QB : 3 * QB], in_=x_d[:, 2 * QB : 3 * QB])
        nc.scalar.dma_start(out=xf[:, 3 * QB : 4 * QB], in_=x_d[:, 3 * QB : 4 * QB])
        nc.gpsimd.dma_start(out=wb16, in_=wb16_d)
        nc.sync.dma_start(out=xf[:, 4 * QB : 5 * QB], in_=x_d[:, 4 * QB : 5 * QB])
        nc.scalar.dma_start(out=xf[:, 5 * QB : 6 * QB], in_=x_d[:, 5 * QB : 6 * QB])
        nc.gpsimd.dma_start(out=xf[:, 6 * QB : 7 * QB], in_=x_d[:, 6 * QB : 7 * QB])
        nc.sync.dma_start(out=xf[:, 7 * QB : 8 * QB], in_=x_d[:, 7 * QB : 8 * QB])
        nc.scalar.dma_start(out=baux, in_=baux_d)

        nc.vector.memset(ones65, 1.0)
        # ones row (partition 80) for the A-matmul bias fold; whole-tile
        # memset (start partition must be 0), pool overwrites rows 0:80
        nc.vector.memset(kvh, 1.0)

        attn0 = attn_pool.tile([KC, NCH * QB], BF16, tag="attn")
        sc0 = [None] * 4

        def late_tail(c):
            # A_c matmul (bias folded via kvh ones row), ab copy, block-0
            # scores; 1024-wide exp per pair on ACT
            csl = slice(c * KC, (c + 1) * KC)
            a_ps = ps_fin.tile([C, QB], F32, tag="fin", name=f"a{c}")
            nc.tensor.matmul(
                a_ps[:, 0:KC], lhsT=w_qt17, rhs=kvh[DV : KV + 1, csl],
                start=True, stop=True, tile_position=(DV, 0),
            )
            nc.vector.tensor_copy(ab[:, csl], a_ps[:, 0:KC])
            t = c // 2
            if c % 2 == 0:
                sc0[t] = ps_sc.tile([KC, 2 * QB], F32, tag="sc", name=f"sc0_{t}")
            nc.tensor.matmul(
                sc0[t][:, (c % 2) * QB : (c % 2 + 1) * QB],
                lhsT=ab[:, csl],
                rhs=xf[:, 0:QB],
                start=True,
                stop=True,
            )
            if c % 2 == 1:
                nc.scalar.activation(
                    attn0[:, (t * 2) * QB : (t * 2 + 2) * QB],
                    sc0[t][:, :],
                    mybir.ActivationFunctionType.Exp,
                )

        # ---- prologue: kv proj + pool chain, block-0 scores interleaved ----
        proj = None
        for c in range(NCH):
            j = c % 2
            if j == 0:
                proj = ps_sc.tile([KC, 2 * QB], F32, tag="sc", name=f"proj{c}")
            sl = slice(c * QB, (c + 1) * QB)
            nc.tensor.matmul(
                proj[:KV, j * QB : (j + 1) * QB],
                lhsT=w_kv,
                rhs=xf[:, sl],
                start=True,
                stop=True,
            )
            csl = slice(c * KC, (c + 1) * KC)
            # maxpool 2x2 via DVE: w-pairs then h-pairs
            pv = proj[:KV, j * QB : (j + 1) * QB].rearrange(
                "p (w two) -> p w two", two=2
            )
            s1 = s1_pool.tile([KV, 256], F32, tag="s1")
            nc.vector.tensor_copy(s1[:, :], pv[:, :, 0])
            nc.vector.tensor_max(s1[:, :], s1[:, :], pv[:, :, 1])
            sv = s1.rearrange("p (h two w) -> p h two w", h=4, two=2)
            nc.vector.tensor_max(kvh[:KV, csl], sv[:, :, 0, :], sv[:, :, 1, :])
            if c >= 1:
                late_tail(c - 1)
        late_tail(NCH - 1)

        ebqk = singles.tile([KC, NCH], F32, tag="ebqk")
        vaug = singles.tile([KC, NCH * (DV + 1)], BF16, tag="vaug")

        def defer_kv_aux():
            # bqk, ebqk, vT transposes, vaug assembly (needed before av(0))
            vt_t = ps_fin.tile([C, QB], F32, tag="fin")  # 8x(128,64) vT chunks
            vt16 = vt_t.bitcast(F16)
            bqk_t = ps_fin.tile([C, QB], F32, tag="fin")  # cols 0:8 used
            for c in range(NCH):
                csl = slice(c * KC, (c + 1) * KC)
                nc.tensor.matmul(
                    bqk_t[:, c : c + 1], lhsT=kvh[DV : DV + D, csl], rhs=b_q64,
                    start=True, stop=True, tile_position=(DV, 0),
                )
                nc.tensor.transpose(
                    vt16[:, c * DV : (c + 1) * DV], kvh[0:DV, csl], identh
                )
            nc.scalar.activation(
                ebqk[:, :], bqk_t[:, 0:NCH],
                mybir.ActivationFunctionType.Exp, bias=bkbq,
            )
            for c in range(NCH):
                base = c * (DV + 1)
                nc.vector.tensor_scalar_mul(
                    vaug[:, base : base + DV],
                    vt16[:, c * DV : (c + 1) * DV],
                    ebqk[:, c : c + 1],
                )
                nc.vector.tensor_copy(
                    vaug[:, base + DV : base + DV + 1], ebqk[:, c : c + 1]
                )

        # ---- main loop: 4-deep software pipeline (block 0 prefilled) ----
        # iter i: PE [sc(i) x8 | av(i-1) x8 | fin(i-3)]
        #         ACT [exp(i) tiles 0-2], DVE [schraudolph exp tile 3,
        #              dn+recip(i-1), ao-mul(i-2), residual-add(i-3)]
        #         DMA [r bounce (i-2), out (i-3)]
        attn_t, av_t, r_t, R65s_t, ao_t = {}, {}, {}, {}, {}
        attn_t[0] = attn0

        for i in range(1, NB + 4):
            b_sc = i          # scores + exp
            b_av = i - 1      # av accumulation + recip
            b_r = i - 2       # broadcast + ao mul
            b_f = i - 3       # fin + residual + store

            if b_sc < NB:
                qsl = slice(b_sc * QB, (b_sc + 1) * QB)
                attn = attn_pool.tile([KC, NCH * QB], BF16, tag="attn")
                attn_t[b_sc] = attn
                attn16 = attn.bitcast(I16)
                for t in range(4):
                    sc = ps_sc.tile([KC, 2 * QB], F32, tag="sc")
                    for j in range(2):
                        cc = 2 * t + j
                        nc.tensor.matmul(
                            sc[:, j * QB : (j + 1) * QB],
                            lhsT=ab[:, cc * KC : (cc + 1) * KC],
                            rhs=xf[:, qsl],
                            start=True,
                            stop=True,
                        )
                    # interleave av MMs of previous block between score tiles
                    if t == 1:
                        if i == 1:
                            defer_kv_aux()
                        elif 0 <= b_av < NB:
                            _av_mms(nc, ps_av, av_t, vaug, attn_t, b_av, 0, 4)
                    if t == 2 and 0 <= b_av < NB:
                        c0 = 0 if i == 1 else 4
                        _av_mms(nc, ps_av, av_t, vaug, attn_t, b_av, c0, 8)
                    if t < 3:
                        nc.scalar.activation(
                            attn[:, t * 2 * QB : (t + 1) * 2 * QB],
                            sc[:, :],
                            mybir.ActivationFunctionType.Exp,
                        )
                    else:
                        # Schraudolph exp on DVE: bf16 bits via int16 affine
                        nc.vector.tensor_scalar(
                            attn16[:, t * 2 * QB : (t + 1) * 2 * QB],
                            sc[:, :],
                            SCH_A,
                            SCH_B,
                            mybir.AluOpType.mult,
                            mybir.AluOpType.add,
                        )
                if b_sc == NB - 1:
                    # last block: start av(7) chunks 0-3 as soon as its first
                    # exps land (rest in the next iteration)
                    _av_mms(nc, ps_av, av_t, vaug, attn_t, b_sc, 0, 4)
            elif 0 <= b_av < NB:
                c0 = 4 if b_av == NB - 1 else 0
                _av_mms(nc, ps_av, av_t, vaug, attn_t, b_av, c0, 8)

            if 0 <= b_av < NB:
                # recip of denominators as soon as av(b_av) stops
                # (custom-DVE recip must read SBUF: stage the psum row first)
                dn = r_pool.tile([1, QB], F32, tag="dn", name=f"dn{b_av}")
                r = r_pool.tile([1, QB], F32, tag="r", name=f"r{b_av}")
                nh = 2 if b_av >= NB - 2 else 1
                for h in range(nh):
                    hs = slice(h * QB // nh, (h + 1) * QB // nh)
                    nc.vector.tensor_copy(dn[:, hs], av_t[b_av][DV : DV + 1, hs])
                    nc.vector.reciprocal_approx_fast(r[:, hs], dn[:, hs])
                r_t[b_av] = r

            if 0 <= b_r < NB:
                R65s = r_pool.tile([DV + 1, QB], F32, tag="R65s", name=f"R65s{b_r}")
                if b_r < NB - 2:
                    # broadcast r over 65 partitions via DRAM bounce (partition
                    # stride 0 on the read); hidden by the 4-deep pipeline
                    r_dram = dram_pool.tile([1, QB], F32, tag="rd", name=f"rd{b_r}")
                    nc.sync.dma_start(out=r_dram[:, :], in_=r_t[b_r][:, :])
                    r_bcast = bass.AP(
                        tensor=r_dram.tensor,
                        offset=r_dram.offset,
                        ap=[[0, DV + 1], [1, QB]],
                    )
                    nc.sync.dma_start(out=R65s[:, :], in_=r_bcast)
                else:
                    # tail blocks: low-latency PE ones-matmul broadcast,
                    # 256-wide halves to pipeline the serial DVE chain
                    rb = r_pool.tile([1, QB], BF16, tag="rb", name=f"rb{b_r}")
                    R65p = ps_fin.tile([C, QB], F32, tag="fin")
                    for h in range(2):
                        hs = slice(h * 256, (h + 1) * 256)
                        nc.vector.tensor_copy(rb[:, hs], r_t[b_r][:, hs])
                        nc.tensor.matmul(
                            R65p[0 : DV + 1, hs], lhsT=ones65, rhs=rb[:, hs],
                            start=True, stop=True,
                        )
                        nc.vector.tensor_copy(R65s[:, hs], R65p[0 : DV + 1, hs])
                R65s_t[b_r] = R65s
                ao = ao_pool.tile([DV + 1, QB], F16, tag="ao")
                ao_t[b_r] = ao
                if b_r >= NB - 2:
                    for h in range(2):
                        hs = slice(h * 256, (h + 1) * 256)
                        nc.vector.tensor_mul(
                            ao[:, hs], av_t[b_r][:, hs], R65s[:, hs]
                        )
                else:
                    nc.vector.tensor_mul(ao[:, :], av_t[b_r][:, :], R65s[:, :])

            if 0 <= b_f < NB:
                qsl = slice(b_f * QB, (b_f + 1) * QB)
                fin = ps_fin.tile([C, QB], F32, tag="fin")
                o = out_pool.tile([C, QB], F32, tag="o")
                if b_f >= NB - 2:
                    for h in range(2):
                        hs = slice(h * 256, (h + 1) * 256)
                        qh = slice(b_f * QB + h * 256, b_f * QB + (h + 1) * 256)
                        nc.tensor.matmul(
                            fin[:, hs], lhsT=w_fin, rhs=ao_t[b_f][:, hs],
                            start=True, stop=True,
                        )
                        nc.vector.tensor_add(o[:, hs], fin[:, hs], xf32[:, qh])
                        nc.sync.dma_start(out=out_d[:, qh], in_=o[:, hs])
                else:
                    nc.tensor.matmul(
                        fin[:, :], lhsT=w_fin, rhs=ao_t[b_f][:, :],
                        start=True, stop=True,
                    )
                    nc.vector.tensor_add(o[:, :], fin[:, :], xf32[:, qsl])
                    nc.sync.dma_start(out=out_d[:, qsl], in_=o[:, :])

    nc.compile()
    return nc


def _av_mms(nc, ps_av, av_t, vaug, attn_t, b, c0, c1):
    if b not in av_t:
        av_t[b] = ps_av.tile([DV + 1, QB], F32, tag="av", name=f"av{b}")
    av = av_t[b]
    attn = attn_t[b]
    for c in range(c0, c1):
        base = c * (DV + 1)
        nc.tensor.matmul(
            av[:, :],
            lhsT=vaug[:, base : base + DV + 1],
            rhs=attn[:, c * QB : (c + 1) * QB],
            start=(c == 0),
            stop=(c == NCH - 1),
        )


def prep_weights(Wq, bq, Wk, bk, Wv, bv, Wo, bo, gamma):
    g = np.float32(np.asarray(gamma))
    Wq, Wk, Wv, Wo = (np.asarray(a, np.float32) for a in (Wq, Wk, Wv, Wo))
    bq_, bk_, bv_, bo_ = (np.asarray(a, np.float32) for a in (bq, bk, bv, bo))
    wb16 = np.zeros((C, W16), np.float16)
    wb16[0:DV, 0:DV] = np.eye(DV, dtype=np.float16)
    wb16[DV : DV + D, DV : DV + C] = Wq.T.astype(np.float16)
    wb16[KV, DV : DV + C] = (Wq @ bk_).astype(np.float16)  # bias row (ones fold)
    wb16[DV : DV + D, DV + C] = bq_.astype(np.float16)
    wb16[0:DV, DV + C + 1 : DV + 2 * C + 1] = (g * Wo).astype(np.float16)
    # bo' = bo + Wo.T bv  (v-bias folded host-side)
    wb16[DV, DV + C + 1 : DV + 2 * C + 1] = (g * (bo_ + Wo.T @ bv_)).astype(
        np.float16
    )
    wb32 = np.zeros((C, W32), np.float32)
    wb32[:, 0:DV] = Wv
    wb32[:, DV : DV + D] = Wk
    baux = np.zeros((C, 1), np.float32)
    baux[:, 0] = np.float32(bk_ @ bq_)  # bqk scalar bias
    return (
        np.ascontiguousarray(wb16),
        np.ascontiguousarray(wb32),
        np.ascontiguousarray(baux),
    )


_NC_CACHE = {}


def kernel(x, Wq, bq, Wk, bk, Wv, bv, Wo, bo, gamma):
    x = np.asarray(x, dtype=np.float32)
    N = x.shape[0]
    assert x.shape == (N, C, 64, 64) and N == NCORES
    wb16, wb32, baux = prep_weights(Wq, bq, Wk, bk, Wv, bv, Wo, bo, gamma)

    if "nc" not in _NC_CACHE:
        _NC_CACHE["nc"] = build_kernel()
    nc = _NC_CACHE["nc"]

    in_maps = []
    for i in range(N):
        in_maps.append(
            {
                "x": np.ascontiguousarray(x[i].reshape(C, HW)),
                "wb16": wb16,
                "wb32": wb32,
                "baux": baux,
            }
        )
    res = bass_utils.run_bass_kernel_spmd(nc, in_maps, core_ids=list(range(N)))
    out = np.stack([res.results[i]["out"].reshape(C, 64, 64) for i in range(N)])
    return out.astype(np.float32)


if __name__ == "__main__":
    print("built", build_kernel())


# revision 8
# speedup vs baseline: 1.0654x; 1.0654x over previous
"""NonLocalAttention2D Trainium2 kernel (v5).

Data-parallel over batch N=8: one image per NeuronCore.

Per-core math (x: (C=128, HW=4096) fp32):
  xh   = fp16(x)                     (128, 4096) GpSimd cast (ACT stays free)
  kv   = [Wv|Wk].T @ xh              (80, 4096)  PE fp16 (v rows 0:64, k 64:80)
  pool = maxpool2x2(kv)              (80, 1024)  DVE max chain -> kvh fp16
  A    = [Wq.T; Wq@bk].T @ [K; 1]    (128, 1024) PE fp16 (bias folded via ones
                                     row 80 of kvh), DVE copy -> ab fp16
  bqk  = k.T @ bq, ebqk = exp(.)     (128, 8)    PE + ACT (bias bk.bq)
  vaugT= [vT*ebqk | ebqk]            (128, 8*65) PE transpose f16 + DVE -> bf16
  s_cb = ab_c.T @ xh_b               (128k,512q) PE fp16 -> psum
  attn = exp(s): tiles 0-2 ACT Exp; tile 3 DVE Schraudolph
         (int16(s*128/ln2 + 16252.5) bitcast bf16, ~2% rel err that cancels
         in the softmax normalization)
  av   = vaugT.T @ attn  (accum 8c)  (65, 512)   PE bf16; row 64 = denom
  r    = recip_approx_fast(denom)    (1, 512)    DVE (input staged to SBUF)
  R65  = broadcast r over 65 parts   DRAM-bounce DMA (PE ones-matmul on tail)
  ao   = av * R65 -> fp16            (65, 512)   DVE
  fin  = [g*Wo; g*bo'].T @ ao        (128, 512)  PE fp16
  out  = fin + x_b                   (128, 512)  DVE -> DMA out

vs v3: x casts run on the idle GpSimd instead of ACT, exp tile 3 of each
block runs on DVE via a Schraudolph bit-trick (so the ACT exp stream no
longer paces the loop), block-0 exps are 1024-wide, biases fold into the
A matmul, and input DMAs dispatch from two engine queues in parallel.
"""

import sys

if "/opt/trn_rl_repo" not in sys.path:
    sys.path.insert(0, "/opt/trn_rl_repo")

import numpy as np

import concourse.bacc as bacc
import concourse.bass as bass
import concourse.tile as tile
from concourse import bass_utils, mybir

F32 = mybir.dt.float32
F16 = mybir.dt.float16
BF16 = mybir.dt.bfloat16
I16 = mybir.dt.int16

C = 128          # channels
HW = 4096        # 64*64 pixels
L = 1024         # pooled keys (32*32)
D = 16           # attn dim
DV = 64          # value dim
KV = 80          # kv projection out width (v rows 0:64, k rows 64:80)
QB = 512         # q-block size
NB = HW // QB    # 8 q blocks
KC = 128         # keys per chunk
NCH = L // KC    # 8 key chunks
NCORES = 8
# wb: ident64 | wqt17 (rows 64:81) | bq (rows 64:80) | wfin | wkv
W16 = DV + C + 1 + C + KV

# Schraudolph exp -> bf16 bits: bits = trunc(s * 2^7/ln2 + (127*2^7 - 4 + 0.5))
SCH_A = 128.0 / 0.6931471805599453
SCH_B = 16252.5


def build_kernel():
    nc = bacc.Bacc("TRN2", target_bir_lowering=False, debug=False)

    x_d = nc.dram_tensor("x", (C, HW), F32, kind="ExternalInput").ap()
    wb_d = nc.dram_tensor("wb", (C, W16), F16, kind="ExternalInput").ap()
    baux_d = nc.dram_tensor("baux", (C, 1), F32, kind="ExternalInput").ap()
    out_d = nc.dram_tensor("out", (C, HW), F32, kind="ExternalOutput").ap()

    from contextlib import ExitStack

    with tile.TileContext(nc) as tc, ExitStack() as ctx:
        singles = ctx.enter_context(tc.tile_pool(name="singles", bufs=1))
        s1_pool = ctx.enter_context(tc.tile_pool(name="s1", bufs=4))
        attn_pool = ctx.enter_context(tc.tile_pool(name="attn", bufs=2))
        r_pool = ctx.enter_context(tc.tile_pool(name="r", bufs=2))
        ao_pool = ctx.enter_context(tc.tile_pool(name="ao", bufs=2))
        out_pool = ctx.enter_context(tc.tile_pool(name="outp", bufs=3))
        dram_pool = ctx.enter_context(tc.tile_pool(name="dram", bufs=2, space="DRAM"))

        ps_sc = ctx.enter_context(tc.tile_pool(name="ps_sc", bufs=2, space="PSUM"))
        ps_av = ctx.enter_context(tc.tile_pool(name="ps_av", bufs=2, space="PSUM"))
        ps_fin = ctx.enter_context(tc.tile_pool(name="ps_fin", bufs=2, space="PSUM"))

        # ---- SBUF singles ----
        wb = singles.tile([C, W16], F16, tag="wb")
        xf = singles.tile([C, HW], F32, tag="xf")
        xh = singles.tile([C, HW], F16, tag="xh")
        kvh = singles.tile([KV + 1, L], F16, tag="kvh")  # v 0:64, k 64:80, ones 80
        ab = singles.tile([C, L], F16, tag="ab")
        ones65 = singles.tile([1, DV + 1], BF16, tag="ones")
        baux = singles.tile([C, 1], F32, tag="baux")

        identh = wb[0:DV, 0:DV]
        w_qt17 = wb[DV : KV + 1, DV : DV + C]            # rows 64:81
        b_q64 = wb[DV : DV + D, DV + C : DV + C + 1]     # rows 64:80
        w_fin = wb[0 : DV + 1, DV + C + 1 : DV + 2 * C + 1]
        w_kv = wb[:, DV + 2 * C + 1 : DV + 2 * C + 1 + KV]
        bkbq = baux[:, 0:1]

        # ---- input DMAs: dispatch from both DMA-capable hardware queues
        # (sync, scalar/ACT) in parallel; piece 0 + weights first ----
        nc.sync.dma_start(out=xf[:, 0:QB], in_=x_d[:, 0:QB])
        nc.scalar.dma_start(out=wb, in_=wb_d)
        nc.sync.dma_start(out=xf[:, 2 * QB : 3 * QB], in_=x_d[:, 2 * QB : 3 * QB])
        nc.scalar.dma_start(out=xf[:, QB : 2 * QB], in_=x_d[:, QB : 2 * QB])
        nc.sync.dma_start(out=xf[:, 4 * QB : 5 * QB], in_=x_d[:, 4 * QB : 5 * QB])
        nc.scalar.dma_start(out=xf[:, 3 * QB : 4 * QB], in_=x_d[:, 3 * QB : 4 * QB])
        nc.sync.dma_start(out=xf[:, 6 * QB : 7 * QB], in_=x_d[:, 6 * QB : 7 * QB])
        nc.scalar.dma_start(out=xf[:, 5 * QB : 6 * QB], in_=x_d[:, 5 * QB : 6 * QB])
        nc.sync.dma_start(out=baux, in_=baux_d)
        nc.scalar.dma_start(out=xf[:, 7 * QB : 8 * QB], in_=x_d[:, 7 * QB : 8 * QB])

        nc.vector.memset(ones65, 1.0)
        # ones row (partition 80) for the A-matmul bias fold; whole-tile
        # memset (start partition must be 0), pool overwrites rows 0:80
        nc.gpsimd.memset(kvh, 1.0)

        def xh_cast(g):  # 512-col pieces on the otherwise-idle GpSimd
            sl = slice(g * QB, (g + 1) * QB)
            nc.gpsimd.tensor_copy(xh[:, sl], xf[:, sl])

        xh_cast(0)
        xh_cast(1)

        attn0 = attn_pool.tile([KC, NCH * QB], BF16, tag="attn")
        sc0 = [None] * 4

        def late_tail(c):
            # A_c matmul (bias folded via kvh ones row), ab copy, block-0
            # scores; 1024-wide exp per pair on ACT
            csl = slice(c * KC, (c + 1) * KC)
            a_ps = ps_fin.tile([C, QB], F32, tag="fin", name=f"a{c}")
            nc.tensor.matmul(
                a_ps[:, 0:KC], lhsT=w_qt17, rhs=kvh[DV : KV + 1, csl],
                start=True, stop=True, tile_position=(DV, 0),
            )
            nc.vector.tensor_copy(ab[:, csl], a_ps[:, 0:KC])
            t = c // 2
            if c % 2 == 0:
                sc0[t] = ps_sc.tile([KC, 2 * QB], F32, tag="sc", name=f"sc0_{t}")
            nc.tensor.matmul(
                sc0[t][:, (c % 2) * QB : (c % 2 + 1) * QB],
                lhsT=ab[:, csl],
                rhs=xh[:, 0:QB],
                start=True,
                stop=True,
            )
            if c % 2 == 1:
                nc.scalar.activation(
                    attn0[:, (t * 2) * QB : (t * 2 + 2) * QB],
                    sc0[t][:, :],
                    mybir.ActivationFunctionType.Exp,
                )

        # ---- prologue: kv proj + pool chain, block-0 scores interleaved ----
        proj = None
        for c in range(NCH):
            j = c % 2
            if j == 0:
                proj = ps_sc.tile([KC, 2 * QB], F32, tag="sc", name=f"proj{c}")
            sl = slice(c * QB, (c + 1) * QB)
            nc.tensor.matmul(
                proj[:KV, j * QB : (j + 1) * QB],
                lhsT=w_kv,
                rhs=xh[:, sl],
                start=True,
                stop=True,
            )
            csl = slice(c * KC, (c + 1) * KC)
            # maxpool 2x2 via DVE: w-pairs then h-pairs
            pv = proj[:KV, j * QB : (j + 1) * QB].rearrange(
                "p (w two) -> p w two", two=2
            )
            if c + 2 < NCH:
                xh_cast(c + 2)  # stay two 512-pieces ahead of proj use
            s1 = s1_pool.tile([KV, 256], F32, tag="s1")
            nc.vector.tensor_copy(s1[:, :], pv[:, :, 0])
            nc.vector.tensor_max(s1[:, :], s1[:, :], pv[:, :, 1])
            sv = s1.rearrange("p (h two w) -> p h two w", h=4, two=2)
            nc.vector.tensor_max(kvh[:KV, csl], sv[:, :, 0, :], sv[:, :, 1, :])
            if c >= 1:
                late_tail(c - 1)
        late_tail(NCH - 1)

        ebqk = singles.tile([KC, NCH], F32, tag="ebqk")
        vaug = singles.tile([KC, NCH * (DV + 1)], BF16, tag="vaug")

        def defer_kv_aux():
            # bqk, ebqk, vT transposes, vaug assembly (needed before av(0))
            vt_t = ps_fin.tile([C, QB], F32, tag="fin")  # 8x(128,64) vT chunks
            vt16 = vt_t.bitcast(F16)
            bqk_t = ps_fin.tile([C, QB], F32, tag="fin")  # cols 0:8 used
            for c in range(NCH):
                csl = slice(c * KC, (c + 1) * KC)
                nc.tensor.matmul(
                    bqk_t[:, c : c + 1], lhsT=kvh[DV : DV + D, csl], rhs=b_q64,
                    start=True, stop=True, tile_position=(DV, 0),
                )
                nc.tensor.transpose(
                    vt16[:, c * DV : (c + 1) * DV], kvh[0:DV, csl], identh
                )
            nc.scalar.activation(
                ebqk[:, :], bqk_t[:, 0:NCH],
                mybir.ActivationFunctionType.Exp, bias=bkbq,
            )
            for c in range(NCH):
                base = c * (DV + 1)
                nc.vector.tensor_scalar_mul(
                    vaug[:, base : base + DV],
                    vt16[:, c * DV : (c + 1) * DV],
                    ebqk[:, c : c + 1],
                )
                nc.vector.tensor_copy(
                    vaug[:, base + DV : base + DV + 1], ebqk[:, c : c + 1]
                )

        # ---- main loop: 4-deep software pipeline (block 0 prefilled) ----
        # iter i: PE [sc(i) x8 | av(i-1) x8 | fin(i-3)]
        #         ACT [exp(i) tiles 0-2], DVE [schraudolph exp tile 3,
        #              dn+recip(i-1), ao-mul(i-2), residual-add(i-3)]
        #         DMA [r bounce (i-2), out (i-3)]
        attn_t, av_t, r_t, R65s_t, ao_t = {}, {}, {}, {}, {}
        attn_t[0] = attn0

        for i in range(1, NB + 4):
            b_sc = i          # scores + exp
            b_av = i - 1      # av accumulation + recip
            b_r = i - 2       # broadcast + ao mul
            b_f = i - 3       # fin + residual + store

            if b_sc < NB:
                qsl = slice(b_sc * QB, (b_sc + 1) * QB)
                attn = attn_pool.tile([KC, NCH * QB], BF16, tag="attn")
                attn_t[b_sc] = attn
                attn16 = attn.bitcast(I16)
                for t in range(4):
                    sc = ps_sc.tile([KC, 2 * QB], F32, tag="sc")
                    for j in range(2):
                        cc = 2 * t + j
                        nc.tensor.matmul(
                            sc[:, j * QB : (j + 1) * QB],
                            lhsT=ab[:, cc * KC : (cc + 1) * KC],
                            rhs=xh[:, qsl],
                            start=True,
                            stop=True,
                        )
                    # interleave av MMs of previous block between score tiles
                    if t == 1:
                        if i == 1:
                            defer_kv_aux()
                        elif 0 <= b_av < NB:
                            _av_mms(nc, ps_av, av_t, vaug, attn_t, b_av, 0, 4)
                    if t == 2 and 0 <= b_av < NB:
                        c0 = 0 if i == 1 else 4
                        _av_mms(nc, ps_av, av_t, vaug, attn_t, b_av, c0, 8)
                    if t < 3:
                        nc.scalar.activation(
                            attn[:, t * 2 * QB : (t + 1) * 2 * QB],
                            sc[:, :],
                            mybir.ActivationFunctionType.Exp,
                        )
                    else:
                        # Schraudolph exp on DVE: bf16 bits via int16 affine
                        nc.vector.tensor_scalar(
                            attn16[:, t * 2 * QB : (t + 1) * 2 * QB],
                            sc[:, :],
                            SCH_A,
                            SCH_B,
                            mybir.AluOpType.mult,
                            mybir.AluOpType.add,
                        )
                if b_sc == NB - 1:
                    # last block: start av(7) chunks 0-3 as soon as its first
                    # exps land (rest in the next iteration)
                    _av_mms(nc, ps_av, av_t, vaug, attn_t, b_sc, 0, 4)
            elif 0 <= b_av < NB:
                c0 = 4 if b_av == NB - 1 else 0
                _av_mms(nc, ps_av, av_t, vaug, attn_t, b_av, c0, 8)

            if 0 <= b_av < NB:
                # recip of denominators as soon as av(b_av) stops
                # (custom-DVE recip must read SBUF: stage the psum row first)
                dn = r_pool.tile([1, QB], F32, tag="dn", name=f"dn{b_av}")
                r = r_pool.tile([1, QB], F32, tag="r", name=f"r{b_av}")
                nh = 2 if b_av >= NB - 2 else 1
                for h in range(nh):
                    hs = slice(h * QB // nh, (h + 1) * QB // nh)
                    nc.vector.tensor_copy(dn[:, hs], av_t[b_av][DV : DV + 1, hs])
                    nc.vector.reciprocal_approx_fast(r[:, hs], dn[:, hs])
                r_t[b_av] = r

            if 0 <= b_r < NB:
                R65s = r_pool.tile([DV + 1, QB], F32, tag="R65s", name=f"R65s{b_r}")
                if b_r < NB - 2:
                    # broadcast r over 65 partitions via DRAM bounce (partition
                    # stride 0 on the read); hidden by the 4-deep pipeline
                    r_dram = dram_pool.tile([1, QB], F32, tag="rd", name=f"rd{b_r}")
                    nc.sync.dma_start(out=r_dram[:, :], in_=r_t[b_r][:, :])
                    r_bcast = bass.AP(
                        tensor=r_dram.tensor,
                        offset=r_dram.offset,
                        ap=[[0, DV + 1], [1, QB]],
                    )
                    nc.sync.dma_start(out=R65s[:, :], in_=r_bcast)
                else:
                    # tail blocks: low-latency PE ones-matmul broadcast,
                    # 256-wide halves to pipeline the serial DVE chain
                    rb = r_pool.tile([1, QB], BF16, tag="rb", name=f"rb{b_r}")
                    R65p = ps_fin.tile([C, QB], F32, tag="fin")
                    for h in range(2):
                        hs = slice(h * 256, (h + 1) * 256)
                        nc.vector.tensor_copy(rb[:, hs], r_t[b_r][:, hs])
                        nc.tensor.matmul(
                            R65p[0 : DV + 1, hs], lhsT=ones65, rhs=rb[:, hs],
                            start=True, stop=True,
                        )
                        nc.vector.tensor_copy(R65s[:, hs], R65p[0 : DV + 1, hs])
                R65s_t[b_r] = R65s
                ao = ao_pool.tile([DV + 1, QB], F16, tag="ao")
                ao_t[b_r] = ao
                if b_r >= NB - 2:
                    for h in range(2):
                        hs = slice(h * 256, (h + 1) * 256)
                        nc.vector.tensor_mul(
                            ao[:, hs], av_t[b_r][:, hs], R65s[:, hs]
                        )
                else:
                    nc.vector.tensor_mul(ao[:, :], av_t[b_r][:, :], R65s[:, :])

            if 0 <= b_f < NB:
                qsl = slice(b_f * QB, (b_f + 1) * QB)
                fin = ps_fin.tile([C, QB], F32, tag="fin")
                o = out_pool.tile([C, QB], F32, tag="o")
                if b_f >= NB - 2:
                    for h in range(2):
                        hs = slice(h * 256, (h + 1) * 256)
                        qh = slice(b_f * QB + h * 256, b_f * QB + (h + 1) * 256)
                        nc.tensor.matmul(
                            fin[:, hs], lhsT=w_fin, rhs=ao_t[b_f][:, hs],
                            start=True, stop=True,
                        )
                        nc.vector.tensor_add(o[:, hs], fin[:, hs], xf[:, qh])
                        nc.sync.dma_start(out=out_d[:, qh], in_=o[:, hs])
                else:
                    nc.tensor.matmul(
                        fin[:, :], lhsT=w_fin, rhs=ao_t[b_f][:, :],
                        start=True, stop=True,
                    )
                    nc.vector.tensor_add(o[:, :], fin[:, :], xf[:, qsl])
                    nc.sync.dma_start(out=out_d[:, qsl], in_=o[:, :])

    nc.compile()
    return nc


def _av_mms(nc, ps_av, av_t, vaug, attn_t, b, c0, c1):
    if b not in av_t:
        av_t[b] = ps_av.tile([DV + 1, QB], F32, tag="av", name=f"av{b}")
    av = av_t[b]
    attn = attn_t[b]
    for c in range(c0, c1):
        base = c * (DV + 1)
        nc.tensor.matmul(
            av[:, :],
            lhsT=vaug[:, base : base + DV + 1],
            rhs=attn[:, c * QB : (c + 1) * QB],
            start=(c == 0),
            stop=(c == NCH - 1),
        )


def prep_weights(Wq, bq, Wk, bk, Wv, bv, Wo, bo, gamma):
    g = np.float32(np.asarray(gamma))
    Wq, Wk, Wv, Wo = (np.asarray(a, np.float32) for a in (Wq, Wk, Wv, Wo))
    bq_, bk_, bv_, bo_ = (np.asarray(a, np.float32) for a in (bq, bk, bv, bo))
    wb = np.zeros((C, W16), np.float16)
    wb[0:DV, 0:DV] = np.eye(DV, dtype=np.float16)
    wb[DV : DV + D, DV : DV + C] = Wq.T.astype(np.float16)
    wb[KV, DV : DV + C] = (Wq @ bk_).astype(np.float16)  # bias row (ones fold)
    wb[DV : DV + D, DV + C] = bq_.astype(np.float16)
    wb[0:DV, DV + C + 1 : DV + 2 * C + 1] = (g * Wo).astype(np.float16)
    # bo' = bo + Wo.T bv  (v-bias folded host-side)
    wb[DV, DV + C + 1 : DV + 2 * C + 1] = (g * (bo_ + Wo.T @ bv_)).astype(
        np.float16
    )
    wb[:, DV + 2 * C + 1 : DV + 2 * C + 1 + DV] = Wv.astype(np.float16)
    wb[:, DV + 2 * C + 1 + DV : DV + 2 * C + 1 + KV] = Wk.astype(np.float16)
    baux = np.zeros((C, 1), np.float32)
    baux[:, 0] = np.float32(bk_ @ bq_)  # bqk scalar bias
    return np.ascontiguousarray(wb), np.ascontiguousarray(baux)


_NC_CACHE = {}


def kernel(x, Wq, bq, Wk, bk, Wv, bv, Wo, bo, gamma):
    x = np.asarray(x, dtype=np.float32)
    N = x.shape[0]
    assert x.shape == (N, C, 64, 64) and N == NCORES
    wb, baux = prep_weights(Wq, bq, Wk, bk, Wv, bv, Wo, bo, gamma)

    if "nc" not in _NC_CACHE:
        _NC_CACHE["nc"] = build_kernel()
    nc = _NC_CACHE["nc"]

    in_maps = []
    for i in range(N):
        in_maps.append(
            {
                "x": np.ascontiguousarray(x[i].reshape(C, HW)),
                "wb": wb,
                "baux": baux,
            }
        )
    res = bass_utils.run_bass_kernel_spmd(nc, in_maps, core_ids=list(range(N)))
    out = np.stack([res.results[i]["out"].reshape(C, 64, 64) for i in range(N)])
    return out.astype(np.float32)


if __name__ == "__main__":
    print("built", build_kernel())
